# revision 36
# baseline (speedup 1.0000x reference)
"""Trainium2 kernel for nn_MeanAbsoluteError_26044681683062.

Reference semantics (per row x[900]):
  1. bandpass: y = irfft(rfft(x) * H), Butterworth-ish magnitude filter, H[0]=0
  2. mean-subtract (exact no-op: H[0]=0)
  3. zero-pad to N=101*900=90900, ps = |FFT|^2 at bins 2020..9090 (40..180 bpm)
  4. argmax over band + 3-point neighbor interpolation -> bpm per row
  5. loss = mean |bpm_pred - bpm_gt|

Steps 1-3 are linear in x, so ps = (x @ Ac)^2 + (x @ As)^2 with
Ac/As = filter-composed cos/sin DFT matrices [900, 7071] (host-precomputed).

Device/host split: the spectrum is 101x oversampled (the peak lobe spans
~100 bins), so the full-resolution argmax can be recovered from a coarse
search.  The 8 cores evaluate the band power on a stride-4 coarse grid
(1768 bins, 221 per core, pure bin-parallel sharding, bf16 matmuls) and
return each core's top-8 coarse values + indices (hardware max/max-index).
The host then evaluates the exact f32 spectrum on a small fine window
around the winning coarse bin (plus any near-tied candidate lobes), takes
the true fine argmax, and applies the reference's 3-point interpolation +
mean — numerically *closer* to the f32 reference than a full device-side
bf16 pipeline.
"""

import os
import sys

import numpy as np
import ml_dtypes

for _p in ("/opt/trn_rl_repo", "/root/.axon_site/_ro/trn_rl_repo"):
    if os.path.isdir(_p) and _p not in sys.path:
        sys.path.append(_p)

import concourse.bass as bass
import concourse.bacc as bacc
import concourse.mybir as mybir
from concourse.tile import TileContext
from concourse.bass_utils import run_bass_kernel_spmd

# ---- problem constants (derived from the reference spec, hardcoded) ----
L = 900              # signal length
B = 512              # 256 preds + 256 gts stacked
FS = 30.0
N = 101 * L          # zero-padded FFT length
LO, HI = 2020, 9091  # band bin range [40,180] bpm on the N-point grid
M = HI - LO          # 7071 band bins
NCORES = 8
STRIDE = 4           # coarse-grid stride (fine bins per coarse bin)
NC_BINS = (M + STRIDE - 1) // STRIDE        # 1768 coarse bins
SLICE = NC_BINS // NCORES                   # 221 coarse bins per core
WIN = 10             # fine half-window the host re-evaluates around a peak
LPAD = 1024          # contraction padded to 8 uniform 128-row chunks
NKCH = LPAD // 128
BF16 = ml_dtypes.bfloat16


def _filter_H():
    freqs = np.fft.rfftfreq(L, d=1.0 / FS).astype(np.float32).astype(np.float64)
    f_safe = freqs + 1e-12
    hp = 1.0 / np.sqrt(1.0 + (0.6 / f_safe) ** 4)
    lp = 1.0 / np.sqrt(1.0 + (f_safe / 4.0) ** 4)
    H = hp * lp
    H[0] = 0.0
    return H


_CACHE = {}


def _prep():
    """Precompute DFT matrices (full f32 for host windows, bf16 coarse
    slabs for the device) and the bpm frequency grid."""
    if _CACHE:
        return _CACHE
    m = np.arange(L)[:, None]
    k = (LO + np.arange(M))[None, :]
    ang = 2.0 * np.pi * ((m * k) % N) / N
    H = _filter_H()[:, None]
    # compose the circulant (symmetric) bandpass with the band DFT
    Ac = np.fft.irfft(np.fft.rfft(np.cos(ang), axis=0) * H, n=L, axis=0)
    As = np.fft.irfft(np.fft.rfft(np.sin(ang), axis=0) * H, n=L, axis=0)
    Ac32 = Ac.astype(np.float32)
    As32 = As.astype(np.float32)

    # coarse-grid slabs: columns at fine bins 0, 4, ..., padded to 8*221
    coarse_c = np.zeros((LPAD, NCORES * SLICE), np.float32)
    coarse_s = np.zeros((LPAD, NCORES * SLICE), np.float32)
    cols = np.arange(NC_BINS) * STRIDE
    coarse_c[:L, :NC_BINS] = Ac32[:, cols]
    coarse_s[:L, :NC_BINS] = As32[:, cols]
    wcs = []
    for c in range(NCORES):
        s = c * SLICE
        both = np.concatenate(
            [coarse_c[:, s:s + SLICE], coarse_s[:, s:s + SLICE]], axis=1)
        wcs.append(np.ascontiguousarray(both).astype(BF16))

    freqs_np = np.fft.fftfreq(N, 1.0 / FS) * 60.0
    _CACHE.update(wcs=wcs, Ac=Ac32, As=As32,
                  freqs=freqs_np[LO:HI].astype(np.float32))
    _CACHE["nc"] = _build_bass()
    return _CACHE


def _build_bass():
    """Bass/Tile program: one NEFF, SPMD across the 8 cores."""
    nc = bacc.Bacc("TRN2", target_bir_lowering=False)
    f32, bf16, u32 = mybir.dt.float32, mybir.dt.bfloat16, mybir.dt.uint32

    xt = nc.dram_tensor("xt", [LPAD, B], bf16, kind="ExternalInput")
    wcs = nc.dram_tensor("wcs", [LPAD, 2 * SLICE], bf16, kind="ExternalInput")
    out_m = nc.dram_tensor("out_m", [B, 8], f32, kind="ExternalOutput")
    out_i = nc.dram_tensor("out_i", [B, 8], u32, kind="ExternalOutput")

    with TileContext(nc) as tc:
        with (
            tc.tile_pool(name="persist", bufs=1) as persist,
            tc.tile_pool(name="work", bufs=4) as work,
            tc.tile_pool(name="psum", bufs=4, space="PSUM") as psum,
        ):
            # PE pre-warm: ~3.4us of sustained dummy matmuls during the DMA
            # phase releases the HAM clock throttle before the real stream
            zk = persist.tile([128, 128], bf16, tag="zk", name="zk")
            nc.gpsimd.memset(zk, 0.0)
            warm = psum.tile([128, 221], f32, tag="pc", name="warm")
            for wi in range(30):
                nc.tensor.matmul(warm[:, 0:128], zk, zk, start=True, stop=True)

            # input DMAs spread across the three DMA-capable queues (SP +
            # ACT HWDGE + GpSimd SWDGE), byte-balanced, k-ordered so the PE
            # can start as soon as the first chunks land
            xt_sb, w_sb = [], []
            engs = [nc.scalar, nc.gpsimd]
            for ki in range(NKCH):
                k0 = ki * 128
                t = persist.tile([128, B], bf16, tag=f"xt{ki}", name=f"xt_sb{ki}")
                nc.sync.dma_start(out=t, in_=xt[k0:k0 + 128, :])
                xt_sb.append(t)
                t = persist.tile([128, 2 * SLICE], bf16, tag=f"w{ki}", name=f"w_sb{ki}")
                engs[ki % 2].dma_start(out=t, in_=wcs[k0:k0 + 128, :])
                w_sb.append(t)

            for mi in range(B // 128):
                m0 = mi * 128
                # cos|sin columns side by side: one accumulation group per
                # m-chunk, [128, 442] f32 still fits a single PSUM bank
                pc = psum.tile([128, 2 * SLICE], f32, tag="pc", name=f"pc_{mi}")
                for ki in range(NKCH):
                    lhsT = xt_sb[ki][:, m0:m0 + 128]
                    nc.tensor.matmul(pc, lhsT, w_sb[ki],
                                     start=ki == 0, stop=ki == NKCH - 1)

                # ps = re^2 + im^2, then hardware top-8 max / max-index
                sq = work.tile([128, 2 * SLICE], f32, tag="sq")
                pst = work.tile([128, SLICE], f32, tag="pst")
                Square = mybir.ActivationFunctionType.Square
                nc.scalar.activation(sq, pc, Square)
                nc.vector.tensor_add(pst, sq[:, 0:SLICE], sq[:, SLICE:2 * SLICE])

                max8 = work.tile([128, 8], f32, tag="max8")
                idx8 = work.tile([128, 8], u32, tag="idx8")
                nc.vector.max(out=max8, in_=pst)
                nc.vector.max_index(out=idx8, in_max=max8, in_values=pst)

                nc.sync.dma_start(out=out_m[m0:m0 + 128, :], in_=max8)
                nc.sync.dma_start(out=out_i[m0:m0 + 128, :], in_=idx8)
    nc.finalize()
    return nc


def _eval_ps(cache, x_row, bins):
    """Exact f32 band power at `bins` for one row (reference-faithful)."""
    Ac, As = cache["Ac"], cache["As"]
    re = x_row @ Ac[:, bins]
    im = x_row @ As[:, bins]
    return re * re + im * im


def kernel(preds: np.ndarray, gts: np.ndarray) -> np.ndarray:
    cache = _prep()
    X = np.concatenate([preds, gts], axis=0).astype(np.float32)
    xt = np.zeros((LPAD, B), BF16)
    xt[:L] = X.T.astype(BF16)

    in_maps = [{"xt": xt, "wcs": cache["wcs"][c]} for c in range(NCORES)]
    res = run_bass_kernel_spmd(
        cache["nc"], in_maps, core_ids=list(range(NCORES)),
        trace=bool(int(os.environ.get("KERNEL_TRACE", "0"))),
    )
    if res.exec_time_ns is not None:
        print(f"HW exec time: {res.exec_time_ns} ns")

    top_v = np.stack([r["out_m"] for r in res.results])   # [8, B, 8] f32
    top_i = np.stack([r["out_i"] for r in res.results])   # [8, B, 8] u32

    # per-row candidate coarse bins: every top-8 entry from every core whose
    # value is within 5% of the row's global coarse max (covers near-tied
    # competitor lobes despite the device's bf16 noise)
    vals = top_v.transpose(1, 0, 2).reshape(B, 64)        # [B, 64]
    gidx = (top_i.transpose(1, 0, 2).astype(np.int64)
            + (np.arange(NCORES) * SLICE)[None, :, None]).reshape(B, 64)
    vmax = vals.max(axis=1)

    freqs = cache["freqs"]
    bpm = np.empty(B, np.float32)
    for b in range(B):
        cand = gidx[b][vals[b] >= 0.95 * vmax[b]]
        fine = np.unique(np.concatenate(
            [np.arange(c * STRIDE - WIN - 1, c * STRIDE + WIN + 2)
             for c in cand]))
        fine = fine[(fine >= 0) & (fine < M)]
        psb = _eval_ps(cache, X[b], fine)
        amax = int(fine[np.argmax(psb)])
        # reference semantics on the fine grid
        if amax == 0:
            bpm[b] = freqs[0]
        elif amax == M - 1:
            bpm[b] = freqs[-1]
        else:
            lut = {int(f): v for f, v in zip(fine, psb)}
            for nb in (amax - 1, amax + 1):
                if nb not in lut:
                    lut[nb] = _eval_ps(cache, X[b], np.array([nb]))[0]
            x1 = np.float32(lut[amax])
            x0 = np.float32(lut[amax - 1])
            x2 = np.float32(lut[amax + 1])
            f0, f1 = freqs[amax - 1], freqs[amax]
            d1 = x1 - x0
            d2 = x1 - x2
            mn = np.minimum(d1, d2)
            mx = np.maximum(d1, d2)
            off = (np.float32(1.0) - mn / mx) * (f1 - f0)
            if d2 > d1:
                off = -off
            bpm[b] = f1 + off

    Bh = B // 2
    return np.asarray(np.mean(np.abs(bpm[:Bh] - bpm[Bh:])), dtype=np.float32)
